# revision 47
# baseline (speedup 1.0000x reference)
"""GNN message-passing kernel for Trainium2 (8 NeuronCores, SPMD).

Reference computation:
    msg  = x[src] * edge_weight[:, None]
    agg  = segment_sum(msg, dst, N) / max(segment_sum(1, dst, N), 1)
    out  = x + alpha * (agg @ W.T + b)

Sharding: nodes are sharded across 8 cores by contiguous ranges; edges are
partitioned by dst so scatter-adds stay local; x (bf16) is replicated to
every core's DRAM so the src-row gather is always local.

Per core, dst nodes are grouped into 128-node blocks. A block's incoming
edges are processed in chunks of 128 edges: a dma_gather fetches the 128
src rows in bf16 (256 B each, one per partition), one DVE tensor_scalar
builds the weighted one-hot Sw[e, j] = w'[e]*(slot[e]==j) in bf16 where
w' = w * alpha / max(deg[dst], 1) is folded on the host, and the tensor
engine accumulates aggT[f, n] += Xg[e, f]^T @ Sw[e, n] in PSUM (f32).

Epilogue per block: Activation evacuates aggT to SBUF bf16; the tensor
engine computes p2[f', n] = W.T^T @ aggT + I @ xrbT (the residual
x + alpha*b is pre-added on the host, transposed to [feat, node] layout,
and accumulated into the same PSUM via an identity matmul); Activation
evacuates p2 to the bf16 output tile. All heavy streams (xrbT, y) are in
transposed [128, nodes] layout so DMA descriptors are large and
contiguous.

dma_gather uses int16 indices, so the src space is split into 4 buckets
of <= 32768 rows; each chunk's edges come from a single (block, bucket)
group. Gathers are batched (one dma_gather per bucket per ~12-block
batch) to amortize the ~1us SWDGE per-instruction overhead; the SWDGE
descriptor ring is enlarged to 4096 entries so descriptor generation
overlaps the transfers.

All 8 cores run one shared program: each core orders its blocks by
descending chunk count and the program uses the per-position max, so the
control flow is identical and only the data differs.
"""

import numpy as np

P = 128
NCORES = 8
NBUCK = 4

# set by test harness for profiling; grading leaves these defaults
TRACE = False
LAST_RESULTS = None
GATHER_BLOCKS = 12      # block-slots per gather batch
MAX_GATHER_CHUNKS = 8   # chunks (x128 idx) per dma_gather call — the HW
                        # SWDGE ring is fixed at 1024 descriptors; larger
                        # calls crash the device regardless of DMA_SCRATCH
DMA_SCRATCH = 16384     # SWDGE ring carveout (bytes/partition; HW default)
TRIM = True             # trim trailing gather descriptors per call
NEG_PAD = False         # -1 pad indices: a small probe showed the HW skips
                        # them in place, but at kernel scale the device
                        # errors (SWDGE ring accounting desync suspected) —
                        # keep 0-padding
USE_NIDX_REG = False    # load per-core valid counts into num_idxs_reg
                        # (only needed with NEG_PAD)


def _to_bf16(a):
    import jax.numpy as jnp
    return np.asarray(jnp.asarray(a, dtype=jnp.bfloat16))


def _preprocess(x, src, dst, w, alpha, b):
    N, D = x.shape
    E = src.shape[0]
    SH = -(-N // NCORES)          # nodes per core shard
    NBLK = -(-SH // P)            # 128-node blocks per core
    SHP = NBLK * P                # padded shard size

    deg = np.bincount(dst, minlength=N).astype(np.float32)
    wp = (w * alpha / np.maximum(deg, 1.0)[dst]).astype(np.float32)

    core = dst // SH
    rel = dst - core * SH
    blk = rel // P
    slot = rel % P

    # bucket boundaries: scan the (a, a, a, N-3a) family for the split that
    # minimizes total shared-schedule chunks (each bucket <= 32767 rows)
    cb = (core * NBLK + blk)
    best = None
    for a in range(25000, 32768, 512):
        sizes = np.array([a, a, a, N - 3 * a])
        if sizes[3] <= 0 or sizes.max() > 32767:
            continue
        bounds = np.concatenate([[0], np.cumsum(sizes)])
        bk = np.searchsorted(bounds, src, side="right") - 1
        cnt = np.bincount(cb * NBUCK + bk, minlength=NCORES * NBLK * NBUCK)
        ch = -(-cnt.reshape(NCORES, NBLK, NBUCK) // P)
        t = ch.sum(axis=2)
        pm = np.argsort(-t, axis=1, kind="stable")
        sc = np.take_along_axis(ch, pm[:, :, None], axis=1)
        total = sc.max(axis=0).sum()
        if best is None or total < best[0]:
            best = (total, bounds)
    bounds = best[1]
    bsizes = np.diff(bounds)
    buck = (np.searchsorted(bounds, src, side="right") - 1).astype(np.int64)

    # per (core, block, bucket) edge counts -> chunk counts
    key = (core * NBLK + blk) * NBUCK + buck
    counts = np.bincount(key, minlength=NCORES * NBLK * NBUCK)
    counts = counts.reshape(NCORES, NBLK, NBUCK)
    chunks = -(-counts // P)                                # [NC, NBLK, NBUCK]
    tot = chunks.sum(axis=2)
    # blocks with no edges still need one (dummy) chunk to init PSUM
    empty = tot == 0
    chunks[:, :, 0] = np.where(empty, 1, chunks[:, :, 0])
    tot = chunks.sum(axis=2)

    perm = np.argsort(-tot, axis=1, kind="stable")          # block order per core
    # shared schedule: per (slot-position, bucket) max chunk count over cores
    sorted_chunks = np.take_along_axis(chunks, perm[:, :, None], axis=1)
    NCH4 = sorted_chunks.max(axis=0)                        # [NBLK, NBUCK]
    # per-position max raw edge count (for trailing-descriptor trim)
    sorted_counts = np.take_along_axis(counts, perm[:, :, None], axis=1)
    MC4 = sorted_counts.max(axis=0)                         # [NBLK, NBUCK]

    # global chunk order: batches of GB block-slots. Per (batch, bucket),
    # groups are packed into gather calls of <= MAX_GATHER_CHUNKS chunks,
    # split at group boundaries, with the call's max-waste group last so the
    # trailing-descriptor trim recovers the most padding.
    GB = GATHER_BLOCKS
    MAXC = MAX_GATHER_CHUNKS

    def a16(v):
        return max(16, -(-v // 16) * 16)

    K_of = np.zeros((NBLK, NBUCK), np.int64)                # chunk start of (s, q)
    batches = []   # (s0, s1, gstart, M, [(q, off, ncols, nidx_full, nidx_trim)])
    kg = 0
    for s0 in range(0, NBLK, GB):
        s1 = min(s0 + GB, NBLK)
        gstart = kg
        calls = []
        for q in range(NBUCK):
            groups = [(s, int(NCH4[s, q]), int(MC4[s, q]))
                      for s in range(s0, s1) if NCH4[s, q] > 0]
            groups.sort(key=lambda g: g[1] * P - a16(g[2]))
            parts, cur, cur_ch = [], [], 0
            for g in groups:
                if cur and cur_ch + g[1] > MAXC:
                    parts.append(cur)
                    cur, cur_ch = [], 0
                cur.append(g)
                cur_ch += g[1]
            if cur:
                parts.append(cur)
            for part in parts:
                mi = max(range(len(part)),
                         key=lambda i: part[i][1] * P - a16(part[i][2]))
                part.append(part.pop(mi))
                off = kg - gstart
                ncols = sum(g[1] for g in part)
                for g in part:
                    K_of[g[0], q] = kg
                    kg += g[1]
                tail = part[-1]
                full = ncols * P
                trimmed = full - (tail[1] * P - a16(tail[2]))
                calls.append((q, off, ncols, full, trimmed))
        batches.append((s0, s1, gstart, kg - gstart, calls))
    C_total = kg
    Mmax = max(M for (_, _, _, M, _) in batches)
    # filler gathers so batches 0/1 initialize every G buffer column with
    # finite data (later trimmed calls leave stale-but-finite bytes)
    batches = [
        (s0, s1, gstart, M,
         calls + ([(0, M, Mmax - M, (Mmax - M) * P, (Mmax - M) * P)]
                  if bi < 2 and M < Mmax and bi + 1 < len(batches) else []))
        for bi, (s0, s1, gstart, M, calls) in enumerate(batches)
    ]

    inv_perm = np.empty_like(perm)
    np.put_along_axis(
        inv_perm, perm,
        np.broadcast_to(np.arange(NBLK), (NCORES, NBLK)).copy(), axis=1)

    # edge placement: flat position = K_of[s, q]*128 + rank within group
    order = np.argsort(key, kind="stable")
    grp_start = np.zeros(NCORES * NBLK * NBUCK, np.int64)
    grp_start[1:] = np.cumsum(counts.ravel())[:-1]
    pos_in_grp = np.arange(E) - grp_start[key[order]]
    co = core[order]
    s_of = inv_perm[co, blk[order]]
    padpos = K_of[s_of, buck[order]] * P + pos_in_grp

    # pad slots get index -1: the HW SWDGE skips negative indices in place
    # (no descriptor, no write), so per-core padding costs nothing. Batches
    # 0/1 pad with 0 instead so every G-buffer byte gets written once with
    # finite data (later skipped/trimmed slots then read stale-but-finite).
    idx_a = np.full((NCORES, C_total * P), -1 if NEG_PAD else 0, np.int16)
    slot_a = np.full((NCORES, C_total * P), 999.0, np.float32)
    w_a = np.zeros((NCORES, C_total * P), np.float32)
    idx_a[co, padpos] = (src[order] - bounds[buck[order]]).astype(np.int16)
    slot_a[co, padpos] = slot[order].astype(np.float32)
    w_a[co, padpos] = wp[order]
    if NEG_PAD and len(batches) > 2:
        # batch 1's filler call reads idx columns into batch 2's region;
        # 0-pad through its furthest read so all its writes are real
        end01 = min(C_total, batches[2][2] + max(0, Mmax - batches[1][3])) * P
        head = idx_a[:, :end01]
        head[head < 0] = 0

    # dma_gather index wrap: index i -> [i % 16, i // 16], replicated to 128
    idx16 = idx_a.reshape(NCORES, C_total * 8, 16).transpose(0, 2, 1)
    idx16 = np.ascontiguousarray(
        np.broadcast_to(idx16[:, None, :, :], (NCORES, 8, 16, C_total * 8))
        .reshape(NCORES, P, C_total * 8))
    # per-chunk columns for tensor_scalar scalars (must be f32 when op0 is
    # is_equal)
    slot_t = np.ascontiguousarray(
        slot_a.reshape(NCORES, C_total, P).transpose(0, 2, 1))
    w_t = np.ascontiguousarray(
        w_a.reshape(NCORES, C_total, P).transpose(0, 2, 1))

    # per-core valid-index counts per gather call (num_idxs_reg): the HW
    # generates descriptors only for non-negative indices, so each core
    # pays for its actual edges, not the shared-schedule maximum
    call_list = []
    for bi, (s0, s1, gstart, M, calls) in enumerate(batches):
        for (q, off, ncols, nfull, ntrim) in calls:
            eff = ntrim if (TRIM and bi >= 2) else nfull
            call_list.append(((gstart + off) * P, eff))
    nidx = np.zeros((NCORES, 1, len(call_list)), np.int32)
    for k, (start, eff) in enumerate(call_list):
        nidx[:, 0, k] = (idx_a[:, start:start + eff] >= 0).sum(axis=1)

    n_core = np.minimum(SH, N - np.arange(NCORES) * SH)
    ids = (np.arange(NCORES)[:, None, None] * SH
           + perm[:, :, None] * P + np.arange(P)[None, None, :])  # [NC, NBLK, P]
    valid = (perm[:, :, None] * P
             + np.arange(P)[None, None, :]) < n_core[:, None, None]
    ids_c = np.where(valid, ids, 0)

    # residual + bias, transposed to [feat, node]: xrbT[c, f, s*128+p]
    xrb = np.zeros((NCORES, NBLK, P, D), np.float32)
    xrb[valid] = x[ids_c[valid]] + alpha * b[None, :]
    xrbT = _to_bf16(
        np.ascontiguousarray(
            xrb.reshape(NCORES, SHP, D).transpose(0, 2, 1)))  # [NC, 128, SHP]

    return dict(
        N=N, D=D, SH=SH, NBLK=NBLK, SHP=SHP, C_total=C_total,
        NCH4=NCH4, MC4=MC4, K_of=K_of, batches=batches, bounds=bounds,
        idx16=idx16, slot_t=slot_t, w_t=w_t, nidx=nidx,
        xrbT=xrbT, ids=ids, valid=valid,
    )


def _build_program(pre, alpha=None):
    import concourse.bacc as bacc
    import concourse.tile as tile
    from concourse import mybir

    f32 = mybir.dt.float32
    bf16 = mybir.dt.bfloat16
    eq = mybir.AluOpType.is_equal
    mult = mybir.AluOpType.mult

    N, NBLK, SHP = pre["N"], pre["NBLK"], pre["SHP"]
    C_total, NCH4, K_of = pre["C_total"], pre["NCH4"], pre["K_of"]
    MC4, bounds = pre["MC4"], pre["bounds"]
    batches = pre["batches"]
    Mmax = max(M for (_, _, _, M, _) in batches)
    Mcap = max(M + sum(n for (_, o, n, _, _) in calls if o >= M)
               for (_, _, _, M, calls) in batches)   # incl. filler columns

    nc = bacc.Bacc(None, target_bir_lowering=False,
                   dynamic_dma_scratch_size=DMA_SCRATCH)
    xw_d = nc.dram_tensor("xw", [N, P], bf16, kind="ExternalInput")
    idx_d = nc.dram_tensor("idx16", [P, C_total * 8], mybir.dt.int16,
                           kind="ExternalInput")
    slot_d = nc.dram_tensor("slot", [P, C_total], f32, kind="ExternalInput")
    wg_d = nc.dram_tensor("wg", [P, C_total], f32, kind="ExternalInput")
    xrbt_d = nc.dram_tensor("xrbt", [P, SHP], bf16, kind="ExternalInput")
    wt_d = nc.dram_tensor("wt", [P, P], bf16, kind="ExternalInput")
    iota_d = nc.dram_tensor("iota", [P, P], bf16, kind="ExternalInput")
    id_d = nc.dram_tensor("ident", [P, P], bf16, kind="ExternalInput")
    ncalls = sum(len(calls) for (_, _, _, _, calls) in batches)
    nidx_d = nc.dram_tensor("nidx", [1, ncalls], mybir.dt.int32,
                            kind="ExternalInput")
    y_d = nc.dram_tensor("y", [P, SHP], bf16, kind="ExternalOutput")

    with tile.TileContext(nc) as tc:
        with (
            tc.tile_pool(name="const", bufs=1) as cpool,
            tc.tile_pool(name="sw", bufs=6) as swpool,
            tc.tile_pool(name="ix", bufs=2) as ixpool,
            tc.tile_pool(name="agg", bufs=3) as aggpool,
            tc.tile_pool(name="xrb", bufs=2) as xrpool,
            tc.tile_pool(name="ot", bufs=2) as otpool,
            tc.tile_pool(name="ps1", bufs=4, space="PSUM") as ps1,
            tc.tile_pool(name="ps2", bufs=2, space="PSUM") as ps2,
        ):
            # gather-critical loads first so the first dma_gather issues
            # as early as possible; bulk constants stream during it
            nidx_s = cpool.tile([1, ncalls], mybir.dt.int32)
            nc.sync.dma_start(out=nidx_s[:], in_=nidx_d[:, :])
            slot_s = cpool.tile([P, C_total], f32)
            w_s = cpool.tile([P, C_total], f32)
            wt_s = cpool.tile([P, P], bf16)
            iota_s = cpool.tile([P, P], bf16)
            id_s = cpool.tile([P, P], bf16)
            nv_reg = nc.gpsimd.alloc_register("nv_reg")
            ci_call = 0

            # manually double-buffered gather destinations
            G2 = [cpool.tile([P, Mcap, P], bf16, name=f"Gbuf{i}")
                  for i in range(2)]

            for bi, (s0, s1, gstart, M, calls) in enumerate(batches):
                nb = s1 - s0
                G = G2[bi % 2]

                # this batch's slice of the gather indices (the filler call
                # of batches 0/1 reads past M into the next batch's region)
                mi = min(Mcap, C_total - gstart)
                ix = ixpool.tile([P, Mcap * 8], mybir.dt.int16)
                nc.sync.dma_start(
                    out=ix[:, :mi * 8],
                    in_=idx_d[:, gstart * 8:(gstart + mi) * 8])

                for (q, off, ncols, nfull, ntrim) in calls:
                    n_idx = ntrim if (TRIM and bi >= 2) else nfull
                    if USE_NIDX_REG:
                        nc.gpsimd.reg_load(
                            nv_reg, nidx_s[0:1, ci_call:ci_call + 1])
                        nreg = nv_reg
                    else:
                        nreg = n_idx
                    ci_call += 1
                    nc.gpsimd.dma_gather(
                        out_ap=G[:, off:off + ncols, :],
                        in_ap=xw_d[int(bounds[q]):int(bounds[q + 1]), :],
                        idxs_ap=ix[:, off * 8:off * 8 + (n_idx + 15) // 16],
                        num_idxs=n_idx,
                        num_idxs_reg=nreg,
                        elem_size=P,
                    )

                if bi == 0:
                    nc.sync.dma_start(out=slot_s[:], in_=slot_d[:, :])
                    nc.sync.dma_start(out=w_s[:], in_=wg_d[:, :])
                    nc.sync.dma_start(out=wt_s[:], in_=wt_d[:, :])
                    nc.sync.dma_start(out=iota_s[:], in_=iota_d[:, :])
                    nc.sync.dma_start(out=id_s[:], in_=id_d[:, :])

                xrt = xrpool.tile([P, nb * P], bf16)
                nc.sync.dma_start(
                    out=xrt[:], in_=xrbt_d[:, s0 * P:s1 * P])
                ot = otpool.tile([P, nb * P], bf16)

                for s in range(s0, s1):
                    nch = int(NCH4[s].sum())
                    p1 = ps1.tile([P, P], f32)
                    ci = 0
                    for q in range(NBUCK):
                        for c in range(int(NCH4[s, q])):
                            k = int(K_of[s, q]) + c
                            sw = swpool.tile([P, P], bf16)
                            nc.vector.tensor_scalar(
                                out=sw[:], in0=iota_s[:],
                                scalar1=slot_s[:, k:k + 1],
                                scalar2=w_s[:, k:k + 1],
                                op0=eq, op1=mult,
                            )
                            nc.tensor.matmul(
                                p1[:], lhsT=G[:, k - gstart, :], rhs=sw[:],
                                start=(ci == 0), stop=(ci == nch - 1),
                            )
                            ci += 1
                    aggT = aggpool.tile([P, P], bf16)
                    nc.scalar.copy(aggT[:], p1[:])
                    j = s - s0
                    p2 = ps2.tile([P, P], f32)
                    nc.tensor.matmul(
                        p2[:], lhsT=wt_s[:], rhs=aggT[:],
                        start=True, stop=False)
                    nc.tensor.matmul(
                        p2[:], lhsT=id_s[:], rhs=xrt[:, j * P:(j + 1) * P],
                        start=False, stop=True)
                    nc.scalar.copy(ot[:, j * P:(j + 1) * P], p2[:])

                nc.sync.dma_start(
                    out=y_d[:, s0 * P:s1 * P], in_=ot[:])

    nc.compile()
    return nc


def kernel(**inputs):
    global LAST_RESULTS
    x = np.ascontiguousarray(np.asarray(inputs["x"], dtype=np.float32))
    ei = np.asarray(inputs["edge_index"])
    w = np.ascontiguousarray(np.asarray(inputs["edge_weight"], dtype=np.float32))
    W = np.asarray(inputs["W"], dtype=np.float32)
    b = np.asarray(inputs["b"], dtype=np.float32)
    alpha = float(np.asarray(inputs["alpha"]))
    src = ei[0].astype(np.int64)
    dst = ei[1].astype(np.int64)

    pre = _preprocess(x, src, dst, w, alpha, b)
    N, D = pre["N"], pre["D"]
    assert D == P

    nc = _build_program(pre)

    xw = _to_bf16(x)
    wt = _to_bf16(np.ascontiguousarray(W.T))
    iota = _to_bf16(np.broadcast_to(
        np.arange(P, dtype=np.float32), (P, P)).copy())
    ident = _to_bf16(np.eye(P, dtype=np.float32))

    in_maps = []
    for c in range(NCORES):
        in_maps.append({
            "xw": xw,
            "idx16": pre["idx16"][c],
            "slot": pre["slot_t"][c],
            "wg": pre["w_t"][c],
            "xrbt": pre["xrbT"][c],
            "wt": wt,
            "iota": iota,
            "ident": ident,
            "nidx": pre["nidx"][c],
        })

    global LAST_NC, LAST_IN_MAPS, LAST_PRE
    LAST_NC, LAST_IN_MAPS, LAST_PRE = nc, in_maps, pre

    from concourse.bass_utils import run_bass_kernel_spmd
    kw = {"trace": True} if TRACE else {}
    res = run_bass_kernel_spmd(
        nc, in_maps, core_ids=list(range(NCORES)), **kw)
    LAST_RESULTS = res

    out = np.empty((N, P), np.float32)
    NBLK = pre["NBLK"]
    valid = pre["valid"]
    ids = pre["ids"]
    for c in range(NCORES):
        yT = np.asarray(res.results[c]["y"]).astype(np.float32)  # [128, SHP]
        y = np.ascontiguousarray(yT.T).reshape(NBLK, P, P)
        out[ids[c][valid[c]]] = y[valid[c]]
    return out
